# revision 28
# baseline (speedup 1.0000x reference)
"""Trainium2 Bass kernel for batched single-head attention.

Problem: x[8, 4096, 512] fp32, Wq/Wk/Wv[512, 256], bq/bk/bv[256].
  Q = x@Wq + bq ; K = x@Wk + bk ; V = x@Wv + bv
  out = softmax(Q K^T / sqrt(256)) V          -> [8, 4096, 256]

Sharding: data-parallel over batch. 8 batch elements -> 8 NeuronCores,
one full attention per core, no collectives. x is cast to bf16 on the
host (input prep) and transposed on-device via PE matmul-with-identity
(a host-side pre-transpose measured slower end-to-end: the projection
phase then stalls on the serial xT DMA stream instead of overlapping
with the transpose matmuls).

All matmuls run in bf16 with fp32 PSUM accumulation (fp32 matmuls on
TRN2 lower to an FP32HI/FP32LO pass pair AND stream the moving operand
at half rate — measured ~4x slower than bf16). Biases are added in fp32
on the PSUM->SBUF copy; softmax row sums / normalization stay fp32.

Per-core algorithm:
  0. xT = x.T via PE matmul-with-identity; 4 chunks per full PSUM bank,
     one strided cast PSUM->SBUF per s-tile, alternating DVE/ACT.
  1. QT/KT [e, s] = W.T @ xT (weights stationary, N=512 moving), bias
     added on the PSUM->SBUF copy via per-partition activation bias.
  2. V [s, e] natural layout (xT chunks stationary), bias via a rank-1
     (K=1) ones @ bv matmul into the same PSUM group. A ones column is
     appended to V so attn@V also yields softmax row sums for free.
  3. Per q-block of 512: scoresT [k, q] = KT.T @ QT block (PE), exp((.)/16)
     on ACT directly PSUM->SBUF (no max subtraction: scores ~ N(0,1), exp
     is fp32-safe), then out[q, 0:257] += PT_chunk.T @ Vext per k-chunk.
     Scores run 2 k-tiles ahead of attn@V (software pipeline) so the PE
     never waits on the ACT exp latency. Normalize with the fp32 row
     sums (col 256) on the DVE on the way out.
"""

import sys

if "/opt/trn_rl_repo" not in sys.path:
    sys.path.insert(0, "/opt/trn_rl_repo")

import ml_dtypes
import numpy as np

import concourse.bass as bass  # noqa: F401
import concourse.mybir as mybir
import concourse.tile as tile
from concourse import bacc
from concourse.bass_utils import run_bass_kernel_spmd

FP32 = mybir.dt.float32
BF16 = mybir.dt.bfloat16
F8 = mybir.dt.float8e4
DR = mybir.MatmulPerfMode.DoubleRow
AF = mybir.ActivationFunctionType

N_CORES = 8
B, S, DIN, D = 8, 4096, 512, 256
P = 128
S_TILES = S // P      # 32 s-tiles
DC = DIN // P         # 4 din chunks
ECH = D // P          # 2 e chunks
QB = 512              # q-block width (columns of scoresT)
N_QB = S // QB        # 8 q-blocks
VE = D + 1            # V columns + ones column = 257
VE_PAD = 260          # padded free extent for the Vext tile
SCALE = 0.0625        # 1/sqrt(256), exact in fp32
EXP_BIAS = -2.0       # shift exp into fp8e4m3 range (max ~240); cancels
                      # exactly in the softmax normalization


def build_program():
    nc = bacc.Bacc(
        "TRN2", target_bir_lowering=False, debug=False, num_devices=N_CORES
    )
    xT_d = nc.dram_tensor("xT", [DIN, S], BF16, kind="ExternalInput")
    wq_d = nc.dram_tensor("Wq", [DIN, D], BF16, kind="ExternalInput")
    bq_d = nc.dram_tensor("bq", [D], FP32, kind="ExternalInput")
    wk_d = nc.dram_tensor("Wk", [DIN, D], BF16, kind="ExternalInput")
    bk_d = nc.dram_tensor("bk", [D], FP32, kind="ExternalInput")
    wv_d = nc.dram_tensor("Wv", [DIN, D], BF16, kind="ExternalInput")
    bv_d = nc.dram_tensor("bv", [D], BF16, kind="ExternalInput")
    out_d = nc.dram_tensor("out", [S, D], FP32, kind="ExternalOutput")

    with tile.TileContext(nc) as tc:
        with (
            tc.tile_pool(name="const", bufs=1) as constp,
            tc.tile_pool(name="big", bufs=1) as bigp,
        ):
            qt = bigp.tile([P, ECH, S], BF16)   # QT: [e-chunk part, ec, s]
            kt = bigp.tile([P, ECH, S], BF16)
            vext = bigp.tile([P, S_TILES, VE_PAD], F8)  # V + ones col, fp8
            xt = bigp.tile([P, DC, S], BF16)    # xT: [din-chunk part, dc, s]
            bv_bc = constp.tile([P, D], BF16)
            ones_row = constp.tile([1, P], BF16)
            expb = constp.tile([P, 1], FP32)
            bv_row = constp.tile([1, D], BF16)
            wq_sb = constp.tile([P, DC, D], BF16)
            wk_sb = constp.tile([P, DC, D], BF16)
            wv_sb = constp.tile([P, DC, D], BF16)
            bqT = constp.tile([P, ECH], FP32)
            bkT = constp.tile([P, ECH], FP32)

            # The Tile framework coarsens each consumer's DMA wait to
            # "every DMA emitted before it in program order" (a per-queue
            # descriptor-count threshold), so DMAs must be emitted
            # just-in-time, interleaved with compute emission. The first
            # Q chain then only waits for wq + x block 0 + bqT on the
            # gpsimd queue (whose preamble ends ~3us before sync's);
            # later weight loads are emitted after the chains that hide
            # them. x arrives pre-transposed from the host ([din, s];
            # contiguous 1KB lines per (chunk, s-block)) - no on-device
            # transpose.
            nc.vector.memset(ones_row[:], 1.0)
            nc.vector.memset(expb[:], EXP_BIAS)
            nc.vector.memset(vext[:, :, D : D + 1], 1.0)
            # wq and x block 0 are split per din-chunk and interleaved so
            # the first chain's dc=0 matmul waits only on the first two
            # small DMAs, not the full 768KB
            xTr = xT_d.rearrange("(c p) s -> p c s", p=P)
            wqr = wq_d.rearrange("(c p) d -> p c d", p=P)
            for c in range(DC):
                nc.sync.dma_start(wq_sb[:, c : c + 1, :], wqr[:, c : c + 1, :])
                nc.sync.dma_start(
                    xt[:, c : c + 1, 0:QB], xTr[:, c : c + 1, 0:QB]
                )
            nc.sync.dma_start(bqT[:], bq_d.rearrange("(c p) -> p c", p=P))

            # PE warmup: dependency-free garbage matmuls (K=1 from the
            # ones row) issued as soon as the engines come up. The PE
            # p-state ramps to full clock only after ~3us of continuous
            # execution; warming it during the initial DMA wait means
            # the first real chains run at full rate.
            with tc.tile_pool(name="warm", bufs=2, space="PSUM") as wp:
                for _ in range(6):
                    pw = wp.tile([P, P], FP32, name="pw")
                    for u in range(4):
                        nc.tensor.matmul(
                            pw[:],
                            ones_row[:],
                            ones_row[:],
                            start=(u == 0),
                            stop=(u == 3),
                        )

            # ---- Phase 1: Q/K/V projections per s-block, consuming xT
            # blocks as their DMAs land. ----
            with (
                tc.tile_pool(name="pjq", bufs=3, space="PSUM") as pjq,
                tc.tile_pool(name="pjv", bufs=2, space="PSUM") as pjv,
            ):
                for sb in range(N_QB):
                    for w_sb, bT, dst in (
                        (wq_sb, bqT, qt),
                        (wk_sb, bkT, kt),
                    ):
                        for ec in range(ECH):
                            ps = pjq.tile([P, QB], FP32)
                            for dc in range(DC):
                                nc.tensor.matmul(
                                    ps[:],
                                    w_sb[:, dc, ec * P : (ec + 1) * P],
                                    xt[:, dc, sb * QB : (sb + 1) * QB],
                                    start=(dc == 0),
                                    stop=(dc == DC - 1),
                                )
                            nc.scalar.activation(
                                dst[:, ec, sb * QB : (sb + 1) * QB],
                                ps[:],
                                AF.Identity,
                                bias=bT[:, ec : ec + 1],
                            )
                        if sb == 0 and w_sb is wq_sb:
                            # K weights: emitted after the Q chains that
                            # hide their load latency
                            nc.gpsimd.dma_start(
                                wk_sb[:],
                                wk_d.rearrange("(c p) d -> p c d", p=P),
                            )
                            nc.gpsimd.dma_start(
                                bkT[:], bk_d.rearrange("(c p) -> p c", p=P)
                            )
                    if sb == 0:
                        nc.gpsimd.dma_start(
                            wv_sb[:], wv_d.rearrange("(c p) d -> p c d", p=P)
                        )
                        nc.gpsimd.dma_start(
                            bv_row[:], bv_d.rearrange("(o d) -> o d", o=1)
                        )
                        # bv broadcast to all partitions (one rank-1
                        # matmul + copy); V tiles then get the bias via a
                        # DVE add on the PSUM->SBUF copy.
                        psb = pjv.tile([P, D], FP32, name="psb", tag="psv")
                        nc.tensor.matmul(
                            psb[:], ones_row[:], bv_row[:],
                            start=True, stop=True,
                        )
                        nc.vector.tensor_copy(bv_bc[:], psb[:])
                    for stv in range(sb * 4, sb * 4 + 4):
                        psv = pjv.tile([P, D], FP32)
                        for dc in range(DC):
                            nc.tensor.matmul(
                                psv[:],
                                xt[:, dc, stv * P : (stv + 1) * P],
                                wv_sb[:, dc, :],
                                start=(dc == 0),
                                stop=(dc == DC - 1),
                            )
                        nc.vector.tensor_add(
                            vext[:, stv, 0:D], psv[:], bv_bc[:]
                        )
                    if sb + 1 < N_QB:
                        # next x block: alternate queues so both DMA
                        # rings pull concurrently; gpsimd first (it
                        # executes after the const loads, not contending
                        # with the sync queue's critical block-0 chunks)
                        xq = nc.gpsimd if sb % 2 == 0 else nc.sync
                        xq.dma_start(
                            xt[:, :, (sb + 1) * QB : (sb + 2) * QB],
                            xTr[:, :, (sb + 1) * QB : (sb + 2) * QB],
                        )

            # ---- Phase 3: attention (software-pipelined: scores run
            # LOOKAHEAD k-tiles ahead of attn@V so the PE never waits on
            # the ACT exp latency). attn@V runs in fp8e4m3 DoubleRow
            # (2 k-tiles contracted per pass at 2x bf16 rate); exp output
            # is written as fp8 pairs [P, 2, QB], shifted by EXP_BIAS to
            # stay in e4m3 range (the shift cancels in normalization). ----
            LOOKAHEAD = 5  # odd: attn@V fires right after a pair completes
            NSTEPS = N_QB * S_TILES
            with (
                tc.tile_pool(name="ptp", bufs=5) as ptp,
                tc.tile_pool(name="accp", bufs=5, space="PSUM") as accp,
                tc.tile_pool(name="scp", bufs=3, space="PSUM") as scp,
                tc.tile_pool(name="outp", bufs=4) as outp,
                tc.tile_pool(name="nrmp", bufs=4) as nrmp,
            ):
                accs = {}
                pts = {}  # pair index -> fp8 tile [P, 2, QB]
                # one flat loop over (q-block, k-tile) so the scores
                # lookahead also spans q-block boundaries
                for step in range(NSTEPS + LOOKAHEAD):
                    if step < NSTEPS:
                        qb, kt_i = divmod(step, S_TILES)
                        if kt_i == 0:
                            accs[qb] = [
                                accp.tile([P, VE], FP32, name="acc", tag="acc")
                                for _ in range(QB // P)
                            ]
                        pss = scp.tile([P, QB], FP32)
                        for ec in range(ECH):
                            nc.tensor.matmul(
                                pss[:],
                                kt[:, ec, kt_i * P : (kt_i + 1) * P],
                                qt[:, ec, qb * QB : (qb + 1) * QB],
                                start=(ec == 0),
                                stop=(ec == ECH - 1),
                            )
                        pair = step // 2
                        if kt_i % 2 == 0:
                            pts[pair] = ptp.tile([P, 2, QB], F8, name="pt2")
                        nc.scalar.activation(
                            pts[pair][:, kt_i % 2, :],
                            pss[:],
                            AF.Exp,
                            bias=expb[:],
                            scale=SCALE,
                        )
                    av = step - LOOKAHEAD
                    if av >= 0 and av % 2 == 1:
                        qb2, kt2 = divmod(av, S_TILES)
                        pav = pts.pop(av // 2)
                        t2 = kt2 - 1  # first k-tile of the pair
                        for j in range(QB // P):
                            nc.tensor.matmul(
                                accs[qb2][j][:],
                                pav[:, :, j * P : (j + 1) * P],
                                vext[:, t2 : t2 + 2, 0:VE],
                                start=(t2 == 0),
                                stop=(t2 == S_TILES - 2),
                                perf_mode=DR,
                            )
                        if kt2 == S_TILES - 1:
                            # normalize alternates DVE/ACT and the output
                            # DMAs alternate sync/gpsimd queues so the
                            # last q-block's drain isn't serialized on a
                            # single engine + queue (kernel tail)
                            for j in range(QB // P):
                                rc = nrmp.tile([P, 1], FP32)
                                nc.vector.reciprocal(
                                    rc[:], accs[qb2][j][:, D : D + 1]
                                )
                                ot = outp.tile([P, D], FP32)
                                # ACT helps only on the last q-block (it
                                # is exp-busy mid-kernel; routing norms
                                # there delays acc PSUM frees otherwise)
                                if qb2 == N_QB - 1 and j % 2 == 1:
                                    nc.scalar.activation(
                                        ot[:],
                                        accs[qb2][j][:, 0:D],
                                        AF.Identity,
                                        scale=rc[:],
                                    )
                                else:
                                    nc.vector.tensor_scalar_mul(
                                        ot[:], accs[qb2][j][:, 0:D], rc[:]
                                    )
                                row = (qb2 * (QB // P) + j) * P
                                dq = nc.sync if j % 2 == 0 else nc.gpsimd
                                dq.dma_start(out_d[row : row + P, :], ot[:])
                            del accs[qb2]

    nc.compile()
    return nc


_NC_CACHE = []


def _get_nc():
    if not _NC_CACHE:
        _NC_CACHE.append(build_program())
    return _NC_CACHE[0]


def kernel(**inputs) -> np.ndarray:
    BF = ml_dtypes.bfloat16
    # ship x pre-transposed ([din, s] per batch): contiguous DMA lines
    # on-device and no PE transpose needed
    xT = np.ascontiguousarray(
        np.asarray(inputs["x"]).astype(BF).transpose(0, 2, 1)
    )
    w = {}
    for k in ("Wq", "Wk", "Wv", "bv"):
        w[k] = np.ascontiguousarray(np.asarray(inputs[k]).astype(BF))
    for k in ("bq", "bk"):
        w[k] = np.ascontiguousarray(np.asarray(inputs[k]).astype(np.float32))
    nc = _get_nc()
    in_maps = [{"xT": xT[b], **w} for b in range(B)]
    res = run_bass_kernel_spmd(nc, in_maps, list(range(N_CORES)))
    return np.stack([res.results[b]["out"] for b in range(B)], axis=0)



# revision 29
# speedup vs baseline: 1.2070x; 1.2070x over previous
"""Trainium2 Bass kernel for batched single-head attention.

Problem: x[8, 4096, 512] fp32, Wq/Wk/Wv[512, 256], bq/bk/bv[256].
  Q = x@Wq + bq ; K = x@Wk + bk ; V = x@Wv + bv
  out = softmax(Q K^T / sqrt(256)) V          -> [8, 4096, 256]

Sharding: data-parallel over batch. 8 batch elements -> 8 NeuronCores,
one full attention per core, no collectives. x is cast to bf16 on the
host (input prep) and transposed on-device via PE matmul-with-identity
(a host-side pre-transpose measured slower end-to-end: the projection
phase then stalls on the serial xT DMA stream instead of overlapping
with the transpose matmuls).

All matmuls run in bf16 with fp32 PSUM accumulation (fp32 matmuls on
TRN2 lower to an FP32HI/FP32LO pass pair AND stream the moving operand
at half rate — measured ~4x slower than bf16). Biases are added in fp32
on the PSUM->SBUF copy; softmax row sums / normalization stay fp32.

Per-core algorithm:
  0. xT = x.T via PE matmul-with-identity; 4 chunks per full PSUM bank,
     one strided cast PSUM->SBUF per s-tile, alternating DVE/ACT.
  1. QT/KT [e, s] = W.T @ xT (weights stationary, N=512 moving), bias
     added on the PSUM->SBUF copy via per-partition activation bias.
  2. V [s, e] natural layout (xT chunks stationary), bias via a rank-1
     (K=1) ones @ bv matmul into the same PSUM group. A ones column is
     appended to V so attn@V also yields softmax row sums for free.
  3. Per q-block of 512: scoresT [k, q] = KT.T @ QT block (PE), exp((.)/16)
     on ACT directly PSUM->SBUF (no max subtraction: scores ~ N(0,1), exp
     is fp32-safe), then out[q, 0:257] += PT_chunk.T @ Vext per k-chunk.
     Scores run 2 k-tiles ahead of attn@V (software pipeline) so the PE
     never waits on the ACT exp latency. Normalize with the fp32 row
     sums (col 256) on the DVE on the way out.
"""

import sys

if "/opt/trn_rl_repo" not in sys.path:
    sys.path.insert(0, "/opt/trn_rl_repo")

import ml_dtypes
import numpy as np

import concourse.bass as bass  # noqa: F401
import concourse.mybir as mybir
import concourse.tile as tile
from concourse import bacc
from concourse.bass_utils import run_bass_kernel_spmd

FP32 = mybir.dt.float32
BF16 = mybir.dt.bfloat16
F8 = mybir.dt.float8e4
DR = mybir.MatmulPerfMode.DoubleRow
AF = mybir.ActivationFunctionType

N_CORES = 8
B, S, DIN, D = 8, 4096, 512, 256
P = 128
S_TILES = S // P      # 32 s-tiles
DC = DIN // P         # 4 din chunks
ECH = D // P          # 2 e chunks
QB = 512              # q-block width (columns of scoresT)
N_QB = S // QB        # 8 q-blocks
VE = D + 1            # V columns + ones column = 257
VE_PAD = 260          # padded free extent for the Vext tile
SCALE = 0.0625        # 1/sqrt(256), exact in fp32
EXP_BIAS = -2.0       # shift exp into fp8e4m3 range (max ~240); cancels
                      # exactly in the softmax normalization


def build_program():
    nc = bacc.Bacc(
        "TRN2", target_bir_lowering=False, debug=False, num_devices=N_CORES
    )
    xT_d = nc.dram_tensor("xT", [DIN, S], BF16, kind="ExternalInput")
    wq_d = nc.dram_tensor("Wq", [DIN, D], BF16, kind="ExternalInput")
    bq_d = nc.dram_tensor("bq", [D], FP32, kind="ExternalInput")
    wk_d = nc.dram_tensor("Wk", [DIN, D], BF16, kind="ExternalInput")
    bk_d = nc.dram_tensor("bk", [D], FP32, kind="ExternalInput")
    wv_d = nc.dram_tensor("Wv", [DIN, D], BF16, kind="ExternalInput")
    bv_d = nc.dram_tensor("bv", [D], BF16, kind="ExternalInput")
    out_d = nc.dram_tensor("out", [S, D], FP32, kind="ExternalOutput")

    with tile.TileContext(nc) as tc:
        with (
            tc.tile_pool(name="const", bufs=1) as constp,
            tc.tile_pool(name="big", bufs=1) as bigp,
        ):
            qt = bigp.tile([P, ECH, S], BF16)   # QT: [e-chunk part, ec, s]
            kt = bigp.tile([P, ECH, S], BF16)
            vext = bigp.tile([P, S_TILES, VE_PAD], F8)  # V + ones col, fp8
            xt = bigp.tile([P, DC, S], BF16)    # xT: [din-chunk part, dc, s]
            bv_bc = constp.tile([P, D], BF16)
            ones_row = constp.tile([1, P], BF16)
            expb = constp.tile([P, 1], FP32)
            bv_row = constp.tile([1, D], BF16)
            wq_sb = constp.tile([P, DC, D], BF16)
            wk_sb = constp.tile([P, DC, D], BF16)
            wv_sb = constp.tile([P, DC, D], BF16)
            bqT = constp.tile([P, ECH], FP32)
            bkT = constp.tile([P, ECH], FP32)

            # The Tile framework coarsens each consumer's DMA wait to
            # "every DMA emitted before it in program order" (a per-queue
            # descriptor-count threshold), so DMAs must be emitted
            # just-in-time, interleaved with compute emission. The first
            # Q chain then only waits for wq + x block 0 + bqT on the
            # gpsimd queue (whose preamble ends ~3us before sync's);
            # later weight loads are emitted after the chains that hide
            # them. x arrives pre-transposed from the host ([din, s];
            # contiguous 1KB lines per (chunk, s-block)) - no on-device
            # transpose.
            nc.vector.memset(ones_row[:], 1.0)
            nc.vector.memset(expb[:], EXP_BIAS)
            nc.vector.memset(vext[:, :, D : D + 1], 1.0)
            # wq and x block 0 are split per din-chunk and interleaved so
            # the first chain's dc=0 matmul waits only on the first two
            # small DMAs, not the full 768KB
            xTr = xT_d.rearrange("(c p) s -> p c s", p=P)
            wqr = wq_d.rearrange("(c p) d -> p c d", p=P)
            for c in range(DC):
                nc.sync.dma_start(wq_sb[:, c : c + 1, :], wqr[:, c : c + 1, :])
                nc.sync.dma_start(
                    xt[:, c : c + 1, 0:QB], xTr[:, c : c + 1, 0:QB]
                )
            nc.sync.dma_start(bqT[:], bq_d.rearrange("(c p) -> p c", p=P))

            # PE warmup: dependency-free garbage matmuls (K=1 from the
            # ones row) issued as soon as the engines come up. The PE
            # p-state ramps to full clock only after ~3us of continuous
            # execution; warming it during the initial DMA wait means
            # the first real chains run at full rate.
            with tc.tile_pool(name="warm", bufs=2, space="PSUM") as wp:
                for _ in range(6):
                    pw = wp.tile([P, P], FP32, name="pw")
                    for u in range(4):
                        nc.tensor.matmul(
                            pw[:],
                            ones_row[:],
                            ones_row[:],
                            start=(u == 0),
                            stop=(u == 3),
                        )

            # ---- Phase 1: Q/K/V projections per s-block, consuming xT
            # blocks as their DMAs land. ----
            with (
                tc.tile_pool(name="pjq", bufs=3, space="PSUM") as pjq,
                tc.tile_pool(name="pjv", bufs=2, space="PSUM") as pjv,
            ):
                for sb in range(N_QB):
                    for w_sb, bT, dst in (
                        (wq_sb, bqT, qt),
                        (wk_sb, bkT, kt),
                    ):
                        for ec in range(ECH):
                            ps = pjq.tile([P, QB], FP32)
                            for dc in range(DC):
                                nc.tensor.matmul(
                                    ps[:],
                                    w_sb[:, dc, ec * P : (ec + 1) * P],
                                    xt[:, dc, sb * QB : (sb + 1) * QB],
                                    start=(dc == 0),
                                    stop=(dc == DC - 1),
                                )
                            nc.scalar.activation(
                                dst[:, ec, sb * QB : (sb + 1) * QB],
                                ps[:],
                                AF.Identity,
                                bias=bT[:, ec : ec + 1],
                            )
                        if sb == 0 and w_sb is wq_sb:
                            # K weights: emitted after the Q chains that
                            # hide their load latency
                            nc.gpsimd.dma_start(
                                wk_sb[:],
                                wk_d.rearrange("(c p) d -> p c d", p=P),
                            )
                            nc.gpsimd.dma_start(
                                bkT[:], bk_d.rearrange("(c p) -> p c", p=P)
                            )
                    if sb == 0:
                        nc.gpsimd.dma_start(
                            wv_sb[:], wv_d.rearrange("(c p) d -> p c d", p=P)
                        )
                        nc.gpsimd.dma_start(
                            bv_row[:], bv_d.rearrange("(o d) -> o d", o=1)
                        )
                        # bv broadcast to all partitions (one rank-1
                        # matmul + copy); V tiles then get the bias via a
                        # DVE add on the PSUM->SBUF copy.
                        psb = pjv.tile([P, D], FP32, name="psb", tag="psv")
                        nc.tensor.matmul(
                            psb[:], ones_row[:], bv_row[:],
                            start=True, stop=True,
                        )
                        nc.vector.tensor_copy(bv_bc[:], psb[:])
                    for stv in range(sb * 4, sb * 4 + 4):
                        psv = pjv.tile([P, D], FP32)
                        for dc in range(DC):
                            nc.tensor.matmul(
                                psv[:],
                                xt[:, dc, stv * P : (stv + 1) * P],
                                wv_sb[:, dc, :],
                                start=(dc == 0),
                                stop=(dc == DC - 1),
                            )
                        nc.vector.tensor_add(
                            vext[:, stv, 0:D], psv[:], bv_bc[:]
                        )
                    if sb + 1 < N_QB:
                        # next x block, split per chunk across both DMA
                        # queues so the two rings pull concurrently and
                        # the block lands in half the time
                        s0, s1 = (sb + 1) * QB, (sb + 2) * QB
                        for c in range(DC):
                            xq = nc.gpsimd if c % 2 == 0 else nc.sync
                            xq.dma_start(
                                xt[:, c : c + 1, s0:s1],
                                xTr[:, c : c + 1, s0:s1],
                            )

            # ---- Phase 3: attention (software-pipelined: scores run
            # LOOKAHEAD k-tiles ahead of attn@V so the PE never waits on
            # the ACT exp latency). attn@V runs in fp8e4m3 DoubleRow
            # (2 k-tiles contracted per pass at 2x bf16 rate); exp output
            # is written as fp8 pairs [P, 2, QB], shifted by EXP_BIAS to
            # stay in e4m3 range (the shift cancels in normalization). ----
            LOOKAHEAD = 5  # odd: attn@V fires right after a pair completes
            NSTEPS = N_QB * S_TILES
            with (
                tc.tile_pool(name="ptp", bufs=5) as ptp,
                tc.tile_pool(name="accp", bufs=5, space="PSUM") as accp,
                tc.tile_pool(name="scp", bufs=3, space="PSUM") as scp,
                tc.tile_pool(name="outp", bufs=4) as outp,
                tc.tile_pool(name="nrmp", bufs=4) as nrmp,
            ):
                accs = {}
                pts = {}  # pair index -> fp8 tile [P, 2, QB]
                # one flat loop over (q-block, k-tile) so the scores
                # lookahead also spans q-block boundaries
                for step in range(NSTEPS + LOOKAHEAD):
                    if step < NSTEPS:
                        qb, kt_i = divmod(step, S_TILES)
                        if kt_i == 0:
                            accs[qb] = [
                                accp.tile([P, VE], FP32, name="acc", tag="acc")
                                for _ in range(QB // P)
                            ]
                        pss = scp.tile([P, QB], FP32)
                        for ec in range(ECH):
                            nc.tensor.matmul(
                                pss[:],
                                kt[:, ec, kt_i * P : (kt_i + 1) * P],
                                qt[:, ec, qb * QB : (qb + 1) * QB],
                                start=(ec == 0),
                                stop=(ec == ECH - 1),
                            )
                        pair = step // 2
                        if kt_i % 2 == 0:
                            pts[pair] = ptp.tile([P, 2, QB], F8, name="pt2")
                        nc.scalar.activation(
                            pts[pair][:, kt_i % 2, :],
                            pss[:],
                            AF.Exp,
                            bias=expb[:],
                            scale=SCALE,
                        )
                    av = step - LOOKAHEAD
                    if av >= 0 and av % 2 == 1:
                        qb2, kt2 = divmod(av, S_TILES)
                        pav = pts.pop(av // 2)
                        t2 = kt2 - 1  # first k-tile of the pair
                        for j in range(QB // P):
                            nc.tensor.matmul(
                                accs[qb2][j][:],
                                pav[:, :, j * P : (j + 1) * P],
                                vext[:, t2 : t2 + 2, 0:VE],
                                start=(t2 == 0),
                                stop=(t2 == S_TILES - 2),
                                perf_mode=DR,
                            )
                        if kt2 == S_TILES - 1:
                            # normalize alternates DVE/ACT and the output
                            # DMAs alternate sync/gpsimd queues so the
                            # last q-block's drain isn't serialized on a
                            # single engine + queue (kernel tail)
                            for j in range(QB // P):
                                rc = nrmp.tile([P, 1], FP32)
                                nc.vector.reciprocal(
                                    rc[:], accs[qb2][j][:, D : D + 1]
                                )
                                ot = outp.tile([P, D], FP32)
                                # ACT helps only on the last q-block (it
                                # is exp-busy mid-kernel; routing norms
                                # there delays acc PSUM frees otherwise)
                                if qb2 == N_QB - 1 and j % 2 == 1:
                                    nc.scalar.activation(
                                        ot[:],
                                        accs[qb2][j][:, 0:D],
                                        AF.Identity,
                                        scale=rc[:],
                                    )
                                else:
                                    nc.vector.tensor_scalar_mul(
                                        ot[:], accs[qb2][j][:, 0:D], rc[:]
                                    )
                                row = (qb2 * (QB // P) + j) * P
                                dq = nc.sync if j % 2 == 0 else nc.gpsimd
                                dq.dma_start(out_d[row : row + P, :], ot[:])
                            del accs[qb2]

    nc.compile()
    return nc


_NC_CACHE = []


def _get_nc():
    if not _NC_CACHE:
        _NC_CACHE.append(build_program())
    return _NC_CACHE[0]


def kernel(**inputs) -> np.ndarray:
    BF = ml_dtypes.bfloat16
    # ship x pre-transposed ([din, s] per batch): contiguous DMA lines
    # on-device and no PE transpose needed
    xT = np.ascontiguousarray(
        np.asarray(inputs["x"]).astype(BF).transpose(0, 2, 1)
    )
    w = {}
    for k in ("Wq", "Wk", "Wv", "bv"):
        w[k] = np.ascontiguousarray(np.asarray(inputs[k]).astype(BF))
    for k in ("bq", "bk"):
        w[k] = np.ascontiguousarray(np.asarray(inputs[k]).astype(np.float32))
    nc = _get_nc()
    in_maps = [{"xT": xT[b], **w} for b in range(B)]
    res = run_bass_kernel_spmd(nc, in_maps, list(range(N_CORES)))
    return np.stack([res.results[b]["out"] for b in range(B)], axis=0)



# revision 31
# speedup vs baseline: 1.2114x; 1.0037x over previous
"""Trainium2 Bass kernel for batched single-head attention.

Problem: x[8, 4096, 512] fp32, Wq/Wk/Wv[512, 256], bq/bk/bv[256].
  Q = x@Wq + bq ; K = x@Wk + bk ; V = x@Wv + bv
  out = softmax(Q K^T / sqrt(256)) V          -> [8, 4096, 256]

Sharding: data-parallel over batch. 8 batch elements -> 8 NeuronCores,
one full attention per core, no collectives. x is cast to bf16 AND
pre-transposed to [din, s] on the host: every on-device consumer wants
xT, the host transpose is free vs ~7us of PE transpose matmuls, and
per-(chunk, s-block) slices are contiguous 1KB DMA lines.

Precision: projections and Q.K^T scores run in bf16 (fp32 PSUM accum).
attn@V runs in fp8 e4m3 DoubleRow perf mode: both P=exp(scores) and V
are quantized to e4m3 and each DR pass contracts 2 k-tiles (K=256) at
1 col/cycle - 2x the bf16 rate (measured; the 0.5 cyc/col in the
CoreSim cost model is NOT what HW does). exp is shifted by -2 so P
fits e4m3's max 240 (the shift cancels in softmax normalization).
Measured end-to-end Frobenius rel err 1.56e-2 (gate 2e-2); bf16
everywhere gives 3.7e-3 but runs ~34us slower. fp8 for scores or the
projections blows the budget (2.6e-2+): exp(16*z) amplifies any
upstream quantization, and hi/lo-split compensation needs 3 DR passes
which loses to 2 bf16 passes at the measured 2x DR rate.

Per-core algorithm:
  1. QT/KT [e, s] = W.T @ xT (weights stationary, N=512 moving), bias
     added on the PSUM->SBUF copy via per-partition activation bias.
  2. V [s, e] natural layout (xT chunks stationary), bias via DVE add
     of a broadcast row (one rank-1 ones @ bv matmul), output in fp8.
     A ones column is appended to V so attn@V also yields softmax row
     sums for free (1.0 is exact in e4m3).
  3. Per q-block of 512: scoresT [k, q] = KT.T @ QT block (bf16), then
     ACT exp PSUM->SBUF into fp8 pair tiles [128, 2, 512]; per k-tile
     PAIR, 4 DoubleRow matmuls accumulate out[q, 0:257] += P.T @ Vext.
     Scores run LOOKAHEAD k-tiles ahead of attn@V (software pipeline)
     so the PE never waits on the ACT exp latency. Normalize with the
     fp32 row sums (col 256) on DVE on the way out.

Schedule notes (the edges are where the time is):
  - Every engine queue has a ~7us NEFF startup preamble and there is a
    ~11us fixed teardown barrier; PE busy (~213us at the fast p-state)
    is within 1% of the cycle-count floor for this algorithm.
  - The Tile framework coarsens DMA waits to "every DMA emitted
    earlier in program order" (per-queue counter threshold), so DMAs
    are emitted just-in-time, interleaved with compute emission.
  - The first Q chain's inputs (wq + x block 0) are split per chunk on
    the sync queue; bulk x blocks prefetch on both queues (chunked,
    alternating) one block ahead; later weight loads hide under
    earlier chains.
  - Dependency-free warmup matmuls ramp the PE p-state (~3us to full
    clock) during the initial DMA wait.
  - The last q-block's normalize alternates DVE/ACT and its output
    DMAs alternate sync/gpsimd so the kernel tail isn't serialized on
    one engine.
  - Run-to-run there are two device clock states (~237us vs ~283us,
    x1.2); nothing in the kernel controls which one a run gets.
"""

import sys

if "/opt/trn_rl_repo" not in sys.path:
    sys.path.insert(0, "/opt/trn_rl_repo")

import ml_dtypes
import numpy as np

import concourse.bass as bass  # noqa: F401
import concourse.mybir as mybir
import concourse.tile as tile
from concourse import bacc
from concourse.bass_utils import run_bass_kernel_spmd

FP32 = mybir.dt.float32
BF16 = mybir.dt.bfloat16
F8 = mybir.dt.float8e4
DR = mybir.MatmulPerfMode.DoubleRow
AF = mybir.ActivationFunctionType

N_CORES = 8
B, S, DIN, D = 8, 4096, 512, 256
P = 128
S_TILES = S // P      # 32 s-tiles
DC = DIN // P         # 4 din chunks
ECH = D // P          # 2 e chunks
QB = 512              # q-block width (columns of scoresT)
N_QB = S // QB        # 8 q-blocks
VE = D + 1            # V columns + ones column = 257
VE_PAD = 260          # padded free extent for the Vext tile
SCALE = 0.0625        # 1/sqrt(256), exact in fp32
EXP_BIAS = -2.0       # shift exp into fp8e4m3 range (max ~240); cancels
                      # exactly in the softmax normalization


def build_program():
    nc = bacc.Bacc(
        "TRN2", target_bir_lowering=False, debug=False, num_devices=N_CORES
    )
    xT_d = nc.dram_tensor("xT", [DIN, S], BF16, kind="ExternalInput")
    wq_d = nc.dram_tensor("Wq", [DIN, D], BF16, kind="ExternalInput")
    bq_d = nc.dram_tensor("bq", [D], FP32, kind="ExternalInput")
    wk_d = nc.dram_tensor("Wk", [DIN, D], BF16, kind="ExternalInput")
    bk_d = nc.dram_tensor("bk", [D], FP32, kind="ExternalInput")
    wv_d = nc.dram_tensor("Wv", [DIN, D], BF16, kind="ExternalInput")
    bv_d = nc.dram_tensor("bv", [D], BF16, kind="ExternalInput")
    out_d = nc.dram_tensor("out", [S, D], FP32, kind="ExternalOutput")

    with tile.TileContext(nc) as tc:
        with (
            tc.tile_pool(name="const", bufs=1) as constp,
            tc.tile_pool(name="big", bufs=1) as bigp,
        ):
            qt = bigp.tile([P, ECH, S], BF16)   # QT: [e-chunk part, ec, s]
            kt = bigp.tile([P, ECH, S], BF16)
            vext = bigp.tile([P, S_TILES, VE_PAD], F8)  # V + ones col, fp8
            xt = bigp.tile([P, DC, S], BF16)    # xT: [din-chunk part, dc, s]
            bv_bc = constp.tile([P, D], BF16)
            ones_row = constp.tile([1, P], BF16)
            expb = constp.tile([P, 1], FP32)
            bv_row = constp.tile([1, D], BF16)
            wq_sb = constp.tile([P, DC, D], BF16)
            wk_sb = constp.tile([P, DC, D], BF16)
            wv_sb = constp.tile([P, DC, D], BF16)
            bqT = constp.tile([P, ECH], FP32)
            bkT = constp.tile([P, ECH], FP32)

            # The Tile framework coarsens each consumer's DMA wait to
            # "every DMA emitted before it in program order" (a per-queue
            # descriptor-count threshold), so DMAs must be emitted
            # just-in-time, interleaved with compute emission. The first
            # Q chain then only waits for wq + x block 0 on the sync
            # queue; later weight loads go on the gpsimd queue after the
            # chains that hide them.
            nc.vector.memset(ones_row[:], 1.0)
            nc.vector.memset(expb[:], EXP_BIAS)
            nc.vector.memset(vext[:, :, D : D + 1], 1.0)
            # wq and x block 0 are split per din-chunk and interleaved so
            # the first chain's dc=0 matmul waits only on the first two
            # small DMAs, not the full 768KB
            xTr = xT_d.rearrange("(c p) s -> p c s", p=P)
            wqr = wq_d.rearrange("(c p) d -> p c d", p=P)
            for c in range(DC):
                nc.sync.dma_start(wq_sb[:, c : c + 1, :], wqr[:, c : c + 1, :])
                nc.sync.dma_start(
                    xt[:, c : c + 1, 0:QB], xTr[:, c : c + 1, 0:QB]
                )
            nc.sync.dma_start(bqT[:], bq_d.rearrange("(c p) -> p c", p=P))

            # PE warmup: dependency-free garbage matmuls (K=1 from the
            # ones row) issued as soon as the engines come up. The PE
            # p-state ramps to full clock only after ~3us of continuous
            # execution; warming it during the initial DMA wait means
            # the first real chains run at full rate.
            with tc.tile_pool(name="warm", bufs=2, space="PSUM") as wp:
                for _ in range(6):
                    pw = wp.tile([P, P], FP32, name="pw")
                    for u in range(4):
                        nc.tensor.matmul(
                            pw[:],
                            ones_row[:],
                            ones_row[:],
                            start=(u == 0),
                            stop=(u == 3),
                        )

            # ---- Phase 1: Q/K/V projections per s-block, consuming xT
            # blocks as their DMAs land. ----
            with (
                tc.tile_pool(name="pjq", bufs=3, space="PSUM") as pjq,
                tc.tile_pool(name="pjv", bufs=2, space="PSUM") as pjv,
            ):
                for sb in range(N_QB):
                    for w_sb, bT, dst in (
                        (wq_sb, bqT, qt),
                        (wk_sb, bkT, kt),
                    ):
                        for ec in range(ECH):
                            ps = pjq.tile([P, QB], FP32)
                            for dc in range(DC):
                                nc.tensor.matmul(
                                    ps[:],
                                    w_sb[:, dc, ec * P : (ec + 1) * P],
                                    xt[:, dc, sb * QB : (sb + 1) * QB],
                                    start=(dc == 0),
                                    stop=(dc == DC - 1),
                                )
                            nc.scalar.activation(
                                dst[:, ec, sb * QB : (sb + 1) * QB],
                                ps[:],
                                AF.Identity,
                                bias=bT[:, ec : ec + 1],
                            )
                        if sb == 0 and w_sb is wq_sb:
                            # K weights: emitted after the Q chains that
                            # hide their load latency
                            nc.gpsimd.dma_start(
                                wk_sb[:],
                                wk_d.rearrange("(c p) d -> p c d", p=P),
                            )
                            nc.gpsimd.dma_start(
                                bkT[:], bk_d.rearrange("(c p) -> p c", p=P)
                            )
                    if sb == 0:
                        nc.gpsimd.dma_start(
                            wv_sb[:], wv_d.rearrange("(c p) d -> p c d", p=P)
                        )
                        nc.gpsimd.dma_start(
                            bv_row[:], bv_d.rearrange("(o d) -> o d", o=1)
                        )
                        # bv broadcast to all partitions (one rank-1
                        # matmul + copy); V tiles then get the bias via a
                        # DVE add on the PSUM->SBUF copy.
                        psb = pjv.tile([P, D], FP32, name="psb", tag="psv")
                        nc.tensor.matmul(
                            psb[:], ones_row[:], bv_row[:],
                            start=True, stop=True,
                        )
                        nc.vector.tensor_copy(bv_bc[:], psb[:])
                    for stv in range(sb * 4, sb * 4 + 4):
                        psv = pjv.tile([P, D], FP32)
                        for dc in range(DC):
                            nc.tensor.matmul(
                                psv[:],
                                xt[:, dc, stv * P : (stv + 1) * P],
                                wv_sb[:, dc, :],
                                start=(dc == 0),
                                stop=(dc == DC - 1),
                            )
                        nc.vector.tensor_add(
                            vext[:, stv, 0:D], psv[:], bv_bc[:]
                        )
                    if sb + 1 < N_QB:
                        # next x block, split per chunk across both DMA
                        # queues so the two rings pull concurrently and
                        # the block lands in half the time
                        s0, s1 = (sb + 1) * QB, (sb + 2) * QB
                        for c in range(DC):
                            xq = nc.gpsimd if c % 2 == 0 else nc.sync
                            xq.dma_start(
                                xt[:, c : c + 1, s0:s1],
                                xTr[:, c : c + 1, s0:s1],
                            )

            # ---- Phase 3: attention (software-pipelined: scores run
            # LOOKAHEAD k-tiles ahead of attn@V so the PE never waits on
            # the ACT exp latency). attn@V runs in fp8e4m3 DoubleRow
            # (2 k-tiles contracted per pass at 2x bf16 rate); exp output
            # is written as fp8 pairs [P, 2, QB], shifted by EXP_BIAS to
            # stay in e4m3 range (the shift cancels in normalization). ----
            LOOKAHEAD = 5  # odd: attn@V fires right after a pair completes
            NSTEPS = N_QB * S_TILES
            with (
                tc.tile_pool(name="ptp", bufs=5) as ptp,
                tc.tile_pool(name="accp", bufs=5, space="PSUM") as accp,
                tc.tile_pool(name="scp", bufs=3, space="PSUM") as scp,
                tc.tile_pool(name="outp", bufs=4) as outp,
                tc.tile_pool(name="nrmp", bufs=4) as nrmp,
            ):
                accs = {}
                pts = {}  # pair index -> fp8 tile [P, 2, QB]
                # one flat loop over (q-block, k-tile) so the scores
                # lookahead also spans q-block boundaries
                for step in range(NSTEPS + LOOKAHEAD):
                    if step < NSTEPS:
                        qb, kt_i = divmod(step, S_TILES)
                        if kt_i == 0:
                            accs[qb] = [
                                accp.tile([P, VE], FP32, name="acc", tag="acc")
                                for _ in range(QB // P)
                            ]
                        pss = scp.tile([P, QB], FP32)
                        for ec in range(ECH):
                            nc.tensor.matmul(
                                pss[:],
                                kt[:, ec, kt_i * P : (kt_i + 1) * P],
                                qt[:, ec, qb * QB : (qb + 1) * QB],
                                start=(ec == 0),
                                stop=(ec == ECH - 1),
                            )
                        pair = step // 2
                        if kt_i % 2 == 0:
                            pts[pair] = ptp.tile([P, 2, QB], F8, name="pt2")
                        nc.scalar.activation(
                            pts[pair][:, kt_i % 2, :],
                            pss[:],
                            AF.Exp,
                            bias=expb[:],
                            scale=SCALE,
                        )
                    av = step - LOOKAHEAD
                    if av >= 0 and av % 2 == 1:
                        qb2, kt2 = divmod(av, S_TILES)
                        pav = pts.pop(av // 2)
                        t2 = kt2 - 1  # first k-tile of the pair
                        for j in range(QB // P):
                            nc.tensor.matmul(
                                accs[qb2][j][:],
                                pav[:, :, j * P : (j + 1) * P],
                                vext[:, t2 : t2 + 2, 0:VE],
                                start=(t2 == 0),
                                stop=(t2 == S_TILES - 2),
                                perf_mode=DR,
                            )
                        if kt2 == S_TILES - 1:
                            # normalize alternates DVE/ACT and the output
                            # DMAs alternate sync/gpsimd queues so the
                            # last q-block's drain isn't serialized on a
                            # single engine + queue (kernel tail)
                            for j in range(QB // P):
                                rc = nrmp.tile([P, 1], FP32)
                                nc.vector.reciprocal(
                                    rc[:], accs[qb2][j][:, D : D + 1]
                                )
                                ot = outp.tile([P, D], FP32)
                                # ACT helps only on the last q-block (it
                                # is exp-busy mid-kernel; routing norms
                                # there delays acc PSUM frees otherwise)
                                if qb2 == N_QB - 1 and j % 2 == 1:
                                    nc.scalar.activation(
                                        ot[:],
                                        accs[qb2][j][:, 0:D],
                                        AF.Identity,
                                        scale=rc[:],
                                    )
                                else:
                                    nc.vector.tensor_scalar_mul(
                                        ot[:], accs[qb2][j][:, 0:D], rc[:]
                                    )
                                row = (qb2 * (QB // P) + j) * P
                                dq = nc.sync if j % 2 == 0 else nc.gpsimd
                                dq.dma_start(out_d[row : row + P, :], ot[:])
                            del accs[qb2]

    nc.compile()
    return nc


_NC_CACHE = []


def _get_nc():
    if not _NC_CACHE:
        _NC_CACHE.append(build_program())
    return _NC_CACHE[0]


def kernel(**inputs) -> np.ndarray:
    BF = ml_dtypes.bfloat16
    # ship x pre-transposed ([din, s] per batch): contiguous DMA lines
    # on-device and no PE transpose needed
    xT = np.ascontiguousarray(
        np.asarray(inputs["x"]).astype(BF).transpose(0, 2, 1)
    )
    w = {}
    for k in ("Wq", "Wk", "Wv", "bv"):
        w[k] = np.ascontiguousarray(np.asarray(inputs[k]).astype(BF))
    for k in ("bq", "bk"):
        w[k] = np.ascontiguousarray(np.asarray(inputs[k]).astype(np.float32))
    nc = _get_nc()
    in_maps = [{"xT": xT[b], **w} for b in range(B)]
    res = run_bass_kernel_spmd(nc, in_maps, list(range(N_CORES)))
    return np.stack([res.results[b]["out"] for b in range(B)], axis=0)

